# revision 21
# baseline (speedup 1.0000x reference)
"""Causal multi-head attention (B=8, L=1024, D_IN=512, H=8, D=64) on 8 TRN2
NeuronCores, data-parallel over batch (one batch element per core, no
collectives).

Per-core layout (batch element b):
  host:   QsT/KsT/VsT = seq[b].T as bf16 [512, 1024]; weights bf16 [512, 512]
          (WQ pre-scaled by 1/sqrt(D) so the softmax scale is free).
  device: qT = WQ.T @ QsT  -> [512(dout), 1024(L)]  (heads on partitions)
          kT likewise; v = (VsT.T @ WV) stored [L, H, 66] with ones columns.
          S^T[j, i] = k_h(j)·q_h(i) per 128-row key tile, exp on ScalarE
          PSUM->SBUF, causal mask = 0/1 multiply on the diagonal 128x128
          block, then O^T[d, i] accumulated over key tiles with
          lhsT = [v_h | 1 | 1] so row 64 carries the softmax denominator.
  host:   OUT[h, :64, :] / OUT[h, 64, :], transpose, concat heads.

A dependency-free chain of dummy matmuls runs first, overlapping the input
DMAs, so the PE's HAM clock gate opens (1.2 -> 2.4 GHz) before real work.
"""

import numpy as np
import ml_dtypes

B, L, D_IN = 8, 1024, 512
H, D = 8, 64
DA = D + 2  # head dim + two ones columns (denominator; padded even so the
# bf16 lhsT slices stay 4-byte aligned — odd column counts hang the HW)
N_CORES = 8
SCALE = 1.0 / np.sqrt(D).item()  # folded into WQ on the host
N_WARMUP = 24  # dummy matmuls to open the HAM clock gate during input DMA

_GRAPH_CACHE = {}


def build_attention_body(tc, qsT, ksT, vsT, wq, wk, wv, mask, out):
    """Emit the per-core kernel into TileContext `tc` (APs per module doc)."""
    import contextlib
    import os

    import concourse.mybir as mybir

    variant = os.environ.get("BASS_ATTN_VARIANT", "full")
    nc = tc.nc
    fp32 = mybir.dt.float32
    bf16 = mybir.dt.bfloat16
    EXP = mybir.ActivationFunctionType.Exp

    with contextlib.ExitStack() as ctx:
        const = ctx.enter_context(tc.tile_pool(name="const", bufs=1))
        sb = ctx.enter_context(tc.tile_pool(name="sb", bufs=1))
        ppool = ctx.enter_context(tc.tile_pool(name="ppool", bufs=5))
        stage = ctx.enter_context(tc.tile_pool(name="stage", bufs=2))
        psum = ctx.enter_context(tc.tile_pool(name="psum", bufs=4, space="PSUM"))

        # ---- PE warm-up: dep-free matmul chain racing the input DMAs ----
        warm_sb = const.tile([128, 512], bf16)
        nc.gpsimd.memset(warm_sb[:], 0.0)
        pwarm = psum.tile([128, 1024], fp32, tag="work", bufs=2, name="pwarm")
        for i in range(N_WARMUP):
            nc.tensor.matmul(
                pwarm[:, 0:512], warm_sb[:, 0:128], warm_sb[:],
                start=True, stop=True,
            )

        # ---- stage inputs into SBUF (ordered by first use) ---------------
        wq_sb = const.tile([128, 4, 512], bf16)
        nc.sync.dma_start(wq_sb[:], wq.rearrange("(kt p) n -> p kt n", p=128))
        wk_sb = const.tile([128, 4, 512], bf16)
        nc.sync.dma_start(wk_sb[:], wk.rearrange("(kt p) n -> p kt n", p=128))
        qsT_sb = const.tile([128, 4, L], bf16)
        nc.sync.dma_start(qsT_sb[:], qsT.rearrange("(kt p) l -> p kt l", p=128))
        ksT_sb = const.tile([128, 4, L], bf16)
        nc.sync.dma_start(ksT_sb[:], ksT.rearrange("(kt p) l -> p kt l", p=128))
        wv_sb = const.tile([128, 4, 512], bf16)
        nc.sync.dma_start(wv_sb[:], wv.rearrange("(kt p) n -> p kt n", p=128))
        vsT_sb = const.tile([128, 4, L], bf16)
        nc.sync.dma_start(vsT_sb[:], vsT.rearrange("(kt p) l -> p kt l", p=128))
        mask_sb = const.tile([128, 128], bf16)
        nc.sync.dma_start(mask_sb[:], mask[:, :])

        # ---- persistent activations -------------------------------------
        qT_sb = sb.tile([128, 4, L], bf16)   # [dout%128, dout//128, L]
        kT_sb = sb.tile([128, 4, L], bf16)
        v_sb = sb.tile([128, 8, H, DA], bf16)  # [j%128, j//128, head, d|1|1]

        def proj_qk_chunk(t, which, nch):
            # one [128, 512] chunk of qT/kT tile t (lhsT = weight tile)
            dst, w_t, src = (
                (qT_sb, wq_sb, qsT_sb), (kT_sb, wk_sb, ksT_sb)
            )[which]
            pq = psum.tile(
                [128, 512], fp32, tag="work", bufs=2,
                name=f"pq_{t}_{which}_{nch}",
            )
            for kt in range(4):
                nc.tensor.matmul(
                    pq[:],
                    w_t[:, kt, t * 128:(t + 1) * 128],
                    src[:, kt, nch * 512:(nch + 1) * 512],
                    start=(kt == 0),
                    stop=(kt == 3),
                )
            nc.vector.tensor_copy(
                out=dst[:, t, nch * 512:(nch + 1) * 512], in_=pq[:]
            )

        def proj_qk(t):
            for which in range(2):
                for nch in range(2):
                    proj_qk_chunk(t, which, nch)

        def proj_v(it):
            # v natural: v[i, n] = sum_k Vs[i, k] WV[k, n]; lhsT = VsT tile
            pv = psum.tile([128, 512], fp32, tag="work", bufs=2, name=f"pv_{it}")
            for kt in range(4):
                nc.tensor.matmul(
                    pv[:],
                    vsT_sb[:, kt, it * 128:(it + 1) * 128],
                    wv_sb[:, kt, :],
                    start=(kt == 0),
                    stop=(kt == 3),
                )
            nc.vector.tensor_copy(
                out=v_sb[:, it, :, 0:D],
                in_=pv.rearrange("p (h d) -> p h d", h=H),
            )
            nc.vector.memset(v_sb[:, it, :, D:DA], 1.0)

        def attention_pair(t):
            oT = [
                psum.tile([DA, L], fp32, tag="oT", bufs=2, name=f"oT_{t}_{hh}")
                for hh in range(2)
            ]
            for jt in range(8):
                # overlap the next pair's qT/kT projection, one 4-matmul
                # chunk per key tile, so it never starves the score slots
                if t < 3 and 2 <= jt <= 5:
                    proj_qk_chunk(t + 1, (jt - 2) // 2, jt % 2)
                j0 = jt * 128
                w = L - j0  # causal: queries i >= j0 only
                for hh in range(2):
                    h = 2 * t + hh
                    pb = 64 * hh  # partition base of this head inside tile t
                    ps = psum.tile(
                        [128, L], fp32, tag="work", bufs=2, name=f"ps_{t}_{jt}_{hh}"
                    )
                    # S^T tile: [key j0..j0+128) x query j0..L); one matmul
                    # per 512-col PSUM bank (f32 matmul out <= 1 bank)
                    for c0 in range(0, w, 512):
                        c1 = min(w, c0 + 512)
                        nc.tensor.matmul(
                            ps[:, c0:c1],
                            kT_sb[pb:pb + 64, t, j0:j0 + 128],
                            qT_sb[pb:pb + 64, t, j0 + c0:j0 + c1],
                            start=True,
                            stop=True,
                        )
                    pexp = ppool.tile([128, L], bf16, tag="P", name=f"P_{t}_{jt}_{hh}")
                    nc.scalar.activation(pexp[:, :w], ps[:, :w], EXP)
                    # causal mask inside the diagonal 128x128 block
                    nc.vector.tensor_mul(pexp[:, 0:128], pexp[:, 0:128], mask_sb[:])
                    # O^T += [v_h | 1].T @ P, per 512-wide PSUM bank
                    for ih in range(2):
                        lo, hi = max(j0, ih * 512), (ih + 1) * 512
                        if lo >= hi:
                            continue
                        nc.tensor.matmul(
                            oT[hh][:, lo:hi],
                            v_sb[:, jt, h, :],
                            pexp[:, lo - j0:hi - j0],
                            start=(jt == 0),
                            stop=(jt == (3 if ih == 0 else 7)),
                            skip_group_check=True,
                        )
            for hh in range(2):
                o_st = stage.tile([DA, L], fp32, tag="ost", name=f"ost_{t}_{hh}")
                nc.vector.tensor_copy(out=o_st[:], in_=oT[hh][:])
                nc.sync.dma_start(out[2 * t + hh], o_st[:])

        # emit: pair-0 dependencies first so the ScalarE exp stream (the
        # critical resource) starts as early as possible
        proj_qk(0)
        for it in range(8):
            proj_v(it)
        if variant == "proj":
            for t in range(1, 4):
                proj_qk(t)
            for h in range(8):
                o_st = stage.tile([DA, L], fp32, tag="ost", name=f"ostp_{h}")
                nc.vector.tensor_copy(out=o_st[:], in_=qT_sb[0:DA, h % 4, :])
                nc.sync.dma_start(out[h], o_st[:])
            return
        for t in range(4):
            attention_pair(t)


def _build_graph():
    import concourse.mybir as mybir
    import concourse.tile as tile
    from concourse import bacc

    nc = bacc.Bacc("TRN2", target_bir_lowering=False)
    bf16 = mybir.dt.bfloat16
    fp32 = mybir.dt.float32
    qsT = nc.dram_tensor("QsT", (D_IN, L), bf16, kind="ExternalInput")
    ksT = nc.dram_tensor("KsT", (D_IN, L), bf16, kind="ExternalInput")
    vsT = nc.dram_tensor("VsT", (D_IN, L), bf16, kind="ExternalInput")
    wq = nc.dram_tensor("WQ", (D_IN, H * D), bf16, kind="ExternalInput")
    wk = nc.dram_tensor("WK", (D_IN, H * D), bf16, kind="ExternalInput")
    wv = nc.dram_tensor("WV", (D_IN, H * D), bf16, kind="ExternalInput")
    mask = nc.dram_tensor("MASK", (128, 128), bf16, kind="ExternalInput")
    out = nc.dram_tensor("OUT", (H, DA, L), fp32, kind="ExternalOutput")

    with tile.TileContext(nc) as tc:
        build_attention_body(
            tc, qsT[:], ksT[:], vsT[:], wq[:], wk[:], wv[:], mask[:], out[:]
        )
    nc.compile()
    return nc


def get_graph():
    if "nc" not in _GRAPH_CACHE:
        _GRAPH_CACHE["nc"] = _build_graph()
    return _GRAPH_CACHE["nc"]


def make_in_maps(Q_seq, K_seq, V_seq, WQ, WK, WV):
    bf = ml_dtypes.bfloat16
    # fold the softmax 1/sqrt(D) into WQ so no scale is needed on-device
    wq = (np.asarray(WQ, dtype=np.float32) * SCALE).astype(bf)
    wk = np.asarray(WK, dtype=np.float32).astype(bf)
    wv = np.asarray(WV, dtype=np.float32).astype(bf)
    # keep-mask in S^T block coords: row r = key offset, col c = query offset;
    # keep key <= query  <=>  r <= c  (upper triangular incl. diagonal)
    mask = np.triu(np.ones((128, 128), dtype=np.float32)).astype(bf)
    in_maps = []
    for b in range(N_CORES):
        in_maps.append({
            "QsT": np.ascontiguousarray(np.asarray(Q_seq[b], np.float32).T).astype(bf),
            "KsT": np.ascontiguousarray(np.asarray(K_seq[b], np.float32).T).astype(bf),
            "VsT": np.ascontiguousarray(np.asarray(V_seq[b], np.float32).T).astype(bf),
            "WQ": wq,
            "WK": wk,
            "WV": wv,
            "MASK": mask,
        })
    return in_maps


def unshard(results):
    """results: list of per-core {"OUT": [H, DA, L] f32} -> [B, L, H*D] f32."""
    outs = np.stack([r["OUT"] for r in results])        # [B, H, DA, L]
    o = outs[:, :, :D, :] / outs[:, :, D:D + 1, :]       # [B, H, D, L]
    return np.ascontiguousarray(
        o.transpose(0, 3, 1, 2).reshape(B, L, H * D)
    ).astype(np.float32)


def run(inputs, **run_kwargs):
    """Compile + run on the 8 cores; returns (output, BassKernelResults)."""
    from concourse.bass_utils import run_bass_kernel_spmd

    nc = get_graph()
    in_maps = make_in_maps(
        inputs["Q_seq"], inputs["K_seq"], inputs["V_seq"],
        inputs["WQ"], inputs["WK"], inputs["WV"],
    )
    res = run_bass_kernel_spmd(
        nc, in_maps, core_ids=list(range(N_CORES)), **run_kwargs
    )
    return unshard(res.results), res


def kernel(Q_seq, K_seq, V_seq, WQ, WK, WV):
    out, _ = run({
        "Q_seq": Q_seq, "K_seq": K_seq, "V_seq": V_seq,
        "WQ": WQ, "WK": WK, "WV": WV,
    })
    return out


# revision 26
# speedup vs baseline: 1.2465x; 1.2465x over previous
"""Causal multi-head attention (B=8, L=1024, D_IN=512, H=8, D=64) on 8 TRN2
NeuronCores, data-parallel over batch (one batch element per core, no
collectives).

Per-core layout (batch element b):
  host:   QsT/KsT/VsT = seq[b].T as bf16 [512, 1024]; weights bf16 [512, 512]
          (WQ pre-scaled by 1/sqrt(D) so the softmax scale is free).
  device: qT = WQ.T @ QsT  -> [512(dout), 1024(L)]  (heads on partitions)
          kT likewise; v = (VsT.T @ WV) stored [L, H, 66] with ones columns.
          S^T[j, i] = k_h(j)·q_h(i) per 128-row key tile, exp on ScalarE
          PSUM->SBUF, causal mask = 0/1 multiply on the diagonal 128x128
          block, then O^T[d, i] accumulated over key tiles with
          lhsT = [v_h | 1 | 1] so row 64 carries the softmax denominator.
  host:   OUT[h, :64, :] / OUT[h, 64, :], transpose, concat heads.

A dependency-free chain of dummy matmuls runs first, overlapping the input
DMAs, so the PE's HAM clock gate opens (1.2 -> 2.4 GHz) before real work.
"""

import numpy as np
import ml_dtypes

B, L, D_IN = 8, 1024, 512
H, D = 8, 64
DA = D + 2  # head dim + two ones columns (denominator; padded even so the
# bf16 lhsT slices stay 4-byte aligned — odd column counts hang the HW)
N_CORES = 8
SCALE = 1.0 / np.sqrt(D).item()  # folded into WQ on the host
N_WARMUP = 24  # dummy matmuls to open the HAM clock gate during input DMA

_GRAPH_CACHE = {}


def build_attention_body(tc, qsT, ksT, vsT, wq, wk, wv, mask, out):
    """Emit the per-core kernel into TileContext `tc` (APs per module doc)."""
    import contextlib
    import os

    import concourse.mybir as mybir

    variant = os.environ.get("BASS_ATTN_VARIANT", "full")
    nc = tc.nc
    fp32 = mybir.dt.float32
    bf16 = mybir.dt.bfloat16
    EXP = mybir.ActivationFunctionType.Exp

    with contextlib.ExitStack() as ctx:
        const = ctx.enter_context(tc.tile_pool(name="const", bufs=1))
        sb = ctx.enter_context(tc.tile_pool(name="sb", bufs=1))
        ppool = ctx.enter_context(tc.tile_pool(name="ppool", bufs=6))
        stage = ctx.enter_context(tc.tile_pool(name="stage", bufs=2))
        psum = ctx.enter_context(tc.tile_pool(name="psum", bufs=4, space="PSUM"))

        # ---- PE warm-up: dep-free matmul chain racing the input DMAs ----
        warm_sb = const.tile([128, 512], bf16)
        nc.gpsimd.memset(warm_sb[:], 0.0)
        pwarm = psum.tile([128, 512], fp32, tag="work", bufs=4, name="pwarm")
        for i in range(N_WARMUP):
            nc.tensor.matmul(
                pwarm[:], warm_sb[:, 0:128], warm_sb[:],
                start=True, stop=True,
            )

        # ---- stage inputs into SBUF (ordered by first use) ---------------
        wq_sb = const.tile([128, 4, 512], bf16)
        nc.sync.dma_start(wq_sb[:], wq.rearrange("(kt p) n -> p kt n", p=128))
        wk_sb = const.tile([128, 4, 512], bf16)
        nc.sync.dma_start(wk_sb[:], wk.rearrange("(kt p) n -> p kt n", p=128))
        qsT_sb = const.tile([128, 4, L], bf16)
        nc.sync.dma_start(qsT_sb[:], qsT.rearrange("(kt p) l -> p kt l", p=128))
        ksT_sb = const.tile([128, 4, L], bf16)
        nc.sync.dma_start(ksT_sb[:], ksT.rearrange("(kt p) l -> p kt l", p=128))
        wv_sb = const.tile([128, 4, 512], bf16)
        nc.sync.dma_start(wv_sb[:], wv.rearrange("(kt p) n -> p kt n", p=128))
        vsT_sb = const.tile([128, 4, L], bf16)
        nc.sync.dma_start(vsT_sb[:], vsT.rearrange("(kt p) l -> p kt l", p=128))
        mask_sb = const.tile([128, 128], bf16)
        nc.sync.dma_start(mask_sb[:], mask[:, :])

        # ---- persistent activations -------------------------------------
        qT_sb = sb.tile([128, 4, L], bf16)   # [dout%128, dout//128, L]
        kT_sb = sb.tile([128, 4, L], bf16)
        v_sb = sb.tile([128, 8, H, DA], bf16)  # [j%128, j//128, head, d|1|1]

        def proj_qk_chunk(t, which, nch):
            # one [128, 512] chunk of qT/kT tile t (lhsT = weight tile)
            dst, w_t, src = (
                (qT_sb, wq_sb, qsT_sb), (kT_sb, wk_sb, ksT_sb)
            )[which]
            pq = psum.tile(
                [128, 512], fp32, tag="work", bufs=4,
                name=f"pq_{t}_{which}_{nch}",
            )
            for kt in range(4):
                nc.tensor.matmul(
                    pq[:],
                    w_t[:, kt, t * 128:(t + 1) * 128],
                    src[:, kt, nch * 512:(nch + 1) * 512],
                    start=(kt == 0),
                    stop=(kt == 3),
                )
            nc.vector.tensor_copy(
                out=dst[:, t, nch * 512:(nch + 1) * 512], in_=pq[:]
            )

        def proj_qk(t):
            for which in range(2):
                for nch in range(2):
                    proj_qk_chunk(t, which, nch)

        def proj_v(it):
            # v natural: v[i, n] = sum_k Vs[i, k] WV[k, n]; lhsT = VsT tile
            pv = psum.tile([128, 512], fp32, tag="work", bufs=4, name=f"pv_{it}")
            for kt in range(4):
                nc.tensor.matmul(
                    pv[:],
                    vsT_sb[:, kt, it * 128:(it + 1) * 128],
                    wv_sb[:, kt, :],
                    start=(kt == 0),
                    stop=(kt == 3),
                )
            nc.vector.tensor_copy(
                out=v_sb[:, it, :, 0:D],
                in_=pv.rearrange("p (h d) -> p h d", h=H),
            )
            nc.vector.memset(v_sb[:, it, :, D:DA], 1.0)

        def attention_pair(t):
            oT = [
                psum.tile([DA, L], fp32, tag="oT", bufs=2, name=f"oT_{t}_{hh}")
                for hh in range(2)
            ]
            for jt in range(8):
                # overlap the next pair's qT/kT projection, one 4-matmul
                # chunk at a time (1 of 4 work slots, never starves scores)
                if t < 3 and 2 <= jt <= 5:
                    proj_qk_chunk(t + 1, (jt - 2) // 2, jt % 2)
                j0 = jt * 128
                for hh in range(2):
                    h = 2 * t + hh
                    pb = 64 * hh  # partition base of this head inside tile t
                    # one [128, <=512] score chunk per oT PSUM bank (ih)
                    for ih in range(2):
                        lo, hi = max(j0, ih * 512), (ih + 1) * 512
                        if lo >= hi:
                            continue
                        cw = hi - lo
                        ps = psum.tile(
                            [128, 512], fp32, tag="work", bufs=4,
                            name=f"ps_{t}_{jt}_{hh}_{ih}",
                        )
                        # S^T chunk: [key j0..j0+128) x query lo..hi)
                        nc.tensor.matmul(
                            ps[:, :cw],
                            kT_sb[pb:pb + 64, t, j0:j0 + 128],
                            qT_sb[pb:pb + 64, t, lo:hi],
                            start=True,
                            stop=True,
                        )
                        pexp = ppool.tile(
                            [128, 512], bf16, tag="P", name=f"P_{t}_{jt}_{hh}_{ih}"
                        )
                        nc.scalar.activation(pexp[:, :cw], ps[:, :cw], EXP)
                        if lo == j0:
                            # causal mask inside the diagonal 128x128 block
                            nc.vector.tensor_mul(
                                pexp[:, 0:128], pexp[:, 0:128], mask_sb[:]
                            )
                        # O^T[:, lo:hi] += [v_h | 1].T @ P
                        nc.tensor.matmul(
                            oT[hh][:, lo:hi],
                            v_sb[:, jt, h, :],
                            pexp[:, :cw],
                            start=(jt == 0),
                            stop=(jt == (3 if ih == 0 else 7)),
                            skip_group_check=True,
                        )
            for hh in range(2):
                o_st = stage.tile([DA, L], fp32, tag="ost", name=f"ost_{t}_{hh}")
                nc.vector.tensor_copy(out=o_st[:], in_=oT[hh][:])
                nc.sync.dma_start(out[2 * t + hh], o_st[:])

        # emit: pair-0 dependencies first so the ScalarE exp stream (the
        # critical resource) starts as early as possible
        proj_qk(0)
        for it in range(8):
            proj_v(it)
        if variant == "proj":
            for t in range(1, 4):
                proj_qk(t)
            for h in range(8):
                o_st = stage.tile([DA, L], fp32, tag="ost", name=f"ostp_{h}")
                nc.vector.tensor_copy(out=o_st[:], in_=qT_sb[0:DA, h % 4, :])
                nc.sync.dma_start(out[h], o_st[:])
            return
        for t in range(4):
            attention_pair(t)


def _build_graph():
    import concourse.mybir as mybir
    import concourse.tile as tile
    from concourse import bacc

    nc = bacc.Bacc("TRN2", target_bir_lowering=False)
    bf16 = mybir.dt.bfloat16
    fp32 = mybir.dt.float32
    qsT = nc.dram_tensor("QsT", (D_IN, L), bf16, kind="ExternalInput")
    ksT = nc.dram_tensor("KsT", (D_IN, L), bf16, kind="ExternalInput")
    vsT = nc.dram_tensor("VsT", (D_IN, L), bf16, kind="ExternalInput")
    wq = nc.dram_tensor("WQ", (D_IN, H * D), bf16, kind="ExternalInput")
    wk = nc.dram_tensor("WK", (D_IN, H * D), bf16, kind="ExternalInput")
    wv = nc.dram_tensor("WV", (D_IN, H * D), bf16, kind="ExternalInput")
    mask = nc.dram_tensor("MASK", (128, 128), bf16, kind="ExternalInput")
    out = nc.dram_tensor("OUT", (H, DA, L), fp32, kind="ExternalOutput")

    with tile.TileContext(nc) as tc:
        build_attention_body(
            tc, qsT[:], ksT[:], vsT[:], wq[:], wk[:], wv[:], mask[:], out[:]
        )
    nc.compile()
    return nc


def get_graph():
    if "nc" not in _GRAPH_CACHE:
        _GRAPH_CACHE["nc"] = _build_graph()
    return _GRAPH_CACHE["nc"]


def make_in_maps(Q_seq, K_seq, V_seq, WQ, WK, WV):
    bf = ml_dtypes.bfloat16
    # fold the softmax 1/sqrt(D) into WQ so no scale is needed on-device
    wq = (np.asarray(WQ, dtype=np.float32) * SCALE).astype(bf)
    wk = np.asarray(WK, dtype=np.float32).astype(bf)
    wv = np.asarray(WV, dtype=np.float32).astype(bf)
    # keep-mask in S^T block coords: row r = key offset, col c = query offset;
    # keep key <= query  <=>  r <= c  (upper triangular incl. diagonal)
    mask = np.triu(np.ones((128, 128), dtype=np.float32)).astype(bf)
    in_maps = []
    for b in range(N_CORES):
        in_maps.append({
            "QsT": np.ascontiguousarray(np.asarray(Q_seq[b], np.float32).T).astype(bf),
            "KsT": np.ascontiguousarray(np.asarray(K_seq[b], np.float32).T).astype(bf),
            "VsT": np.ascontiguousarray(np.asarray(V_seq[b], np.float32).T).astype(bf),
            "WQ": wq,
            "WK": wk,
            "WV": wv,
            "MASK": mask,
        })
    return in_maps


def unshard(results):
    """results: list of per-core {"OUT": [H, DA, L] f32} -> [B, L, H*D] f32."""
    outs = np.stack([r["OUT"] for r in results])        # [B, H, DA, L]
    o = outs[:, :, :D, :] / outs[:, :, D:D + 1, :]       # [B, H, D, L]
    return np.ascontiguousarray(
        o.transpose(0, 3, 1, 2).reshape(B, L, H * D)
    ).astype(np.float32)


def run(inputs, **run_kwargs):
    """Compile + run on the 8 cores; returns (output, BassKernelResults)."""
    from concourse.bass_utils import run_bass_kernel_spmd

    nc = get_graph()
    in_maps = make_in_maps(
        inputs["Q_seq"], inputs["K_seq"], inputs["V_seq"],
        inputs["WQ"], inputs["WK"], inputs["WV"],
    )
    res = run_bass_kernel_spmd(
        nc, in_maps, core_ids=list(range(N_CORES)), **run_kwargs
    )
    return unshard(res.results), res


def kernel(Q_seq, K_seq, V_seq, WQ, WK, WV):
    out, _ = run({
        "Q_seq": Q_seq, "K_seq": K_seq, "V_seq": V_seq,
        "WQ": WQ, "WK": WK, "WV": WV,
    })
    return out


# revision 32
# speedup vs baseline: 1.2965x; 1.0401x over previous
"""Causal multi-head attention (B=8, L=1024, D_IN=512, H=8, D=64) on 8 TRN2
NeuronCores, data-parallel over batch (one batch element per core, no
collectives).

Per-core layout (batch element b):
  host:   QsT/KsT/VsT = seq[b].T as bf16 [512, 1024]; weights bf16 [512, 512]
          (WQ pre-scaled by 1/sqrt(D) so the softmax scale is free).
  device: qT = WQ.T @ QsT  -> [512(dout), 1024(L)]  (heads on partitions)
          kT likewise; v = (VsT.T @ WV) stored [L, H, 66] with ones columns.
          S^T[j, i] = k_h(j)·q_h(i) per 128-row key tile, exp on ScalarE
          PSUM->SBUF, causal mask = 0/1 multiply on the diagonal 128x128
          block, then O^T[d, i] accumulated over key tiles with
          lhsT = [v_h | 1 | 1] so row 64 carries the softmax denominator.
  host:   OUT[h, :64, :] / OUT[h, 64, :], transpose, concat heads.

A dependency-free chain of dummy matmuls runs first, overlapping the input
DMAs, so the PE's HAM clock gate opens (1.2 -> 2.4 GHz) before real work.
"""

import numpy as np
import ml_dtypes

B, L, D_IN = 8, 1024, 512
H, D = 8, 64
DA = D + 2  # head dim + two ones columns (denominator; padded even so the
# bf16 lhsT slices stay 4-byte aligned — odd column counts hang the HW)
N_CORES = 8
SCALE = 1.0 / np.sqrt(D).item()  # folded into WQ on the host
N_WARMUP = 12  # dummy matmuls to open the HAM clock gate during input DMA

_GRAPH_CACHE = {}


def build_attention_body(tc, qsT, ksT, vsT, wq, wk, wv, mask, out):
    """Emit the per-core kernel into TileContext `tc` (APs per module doc)."""
    import contextlib
    import os

    import concourse.mybir as mybir

    variant = os.environ.get("BASS_ATTN_VARIANT", "full")
    nc = tc.nc
    fp32 = mybir.dt.float32
    bf16 = mybir.dt.bfloat16
    EXP = mybir.ActivationFunctionType.Exp

    with contextlib.ExitStack() as ctx:
        const = ctx.enter_context(tc.tile_pool(name="const", bufs=1))
        sb = ctx.enter_context(tc.tile_pool(name="sb", bufs=1))
        ppool = ctx.enter_context(tc.tile_pool(name="ppool", bufs=6))
        stage = ctx.enter_context(tc.tile_pool(name="stage", bufs=2))
        psum = ctx.enter_context(tc.tile_pool(name="psum", bufs=4, space="PSUM"))

        # ---- PE warm-up: dep-free matmul chain racing the input DMAs ----
        warm_sb = const.tile([128, 512], bf16)
        nc.gpsimd.memset(warm_sb[:], 0.0)
        pwarm = psum.tile([128, 512], fp32, tag="work", bufs=4, name="pwarm")
        for i in range(N_WARMUP):
            nc.tensor.matmul(
                pwarm[:], warm_sb[:, 0:128], warm_sb[:],
                start=True, stop=True,
            )

        # ---- stage inputs into SBUF (ordered by first use) ---------------
        wv_sb = const.tile([128, 4, 512], bf16)
        nc.sync.dma_start(wv_sb[:], wv.rearrange("(kt p) n -> p kt n", p=128))
        vsT_sb = const.tile([128, 4, L], bf16)
        nc.sync.dma_start(vsT_sb[:], vsT.rearrange("(kt p) l -> p kt l", p=128))
        wq_sb = const.tile([128, 4, 512], bf16)
        nc.sync.dma_start(wq_sb[:], wq.rearrange("(kt p) n -> p kt n", p=128))
        qsT_sb = const.tile([128, 4, L], bf16)
        nc.sync.dma_start(qsT_sb[:], qsT.rearrange("(kt p) l -> p kt l", p=128))
        wk_sb = const.tile([128, 4, 512], bf16)
        nc.sync.dma_start(wk_sb[:], wk.rearrange("(kt p) n -> p kt n", p=128))
        ksT_sb = const.tile([128, 4, L], bf16)
        nc.sync.dma_start(ksT_sb[:], ksT.rearrange("(kt p) l -> p kt l", p=128))
        mask_sb = const.tile([128, 128], bf16)
        nc.sync.dma_start(mask_sb[:], mask[:, :])

        # ---- persistent activations -------------------------------------
        qT_sb = sb.tile([128, 4, L], bf16)   # [dout%128, dout//128, L]
        kT_sb = sb.tile([128, 4, L], bf16)
        v_sb = sb.tile([128, 8, H, DA], bf16)  # [j%128, j//128, head, d|1|1]

        def proj_qk_chunk(t, which, nch):
            # one [128, 512] chunk of qT/kT tile t (lhsT = weight tile)
            dst, w_t, src = (
                (qT_sb, wq_sb, qsT_sb), (kT_sb, wk_sb, ksT_sb)
            )[which]
            pq = psum.tile(
                [128, 512], fp32, tag="work", bufs=4,
                name=f"pq_{t}_{which}_{nch}",
            )
            for kt in range(4):
                nc.tensor.matmul(
                    pq[:],
                    w_t[:, kt, t * 128:(t + 1) * 128],
                    src[:, kt, nch * 512:(nch + 1) * 512],
                    start=(kt == 0),
                    stop=(kt == 3),
                )
            nc.vector.tensor_copy(
                out=dst[:, t, nch * 512:(nch + 1) * 512], in_=pq[:]
            )

        def proj_qk(t):
            for which in range(2):
                for nch in range(2):
                    proj_qk_chunk(t, which, nch)

        def proj_v(it):
            # v natural: v[i, n] = sum_k Vs[i, k] WV[k, n]; lhsT = VsT tile
            pv = psum.tile([128, 512], fp32, tag="work", bufs=4, name=f"pv_{it}")
            for kt in range(4):
                nc.tensor.matmul(
                    pv[:],
                    vsT_sb[:, kt, it * 128:(it + 1) * 128],
                    wv_sb[:, kt, :],
                    start=(kt == 0),
                    stop=(kt == 3),
                )
            nc.vector.tensor_copy(
                out=v_sb[:, it, :, 0:D],
                in_=pv.rearrange("p (h d) -> p h d", h=H),
            )
            nc.vector.memset(v_sb[:, it, :, D:DA], 1.0)

        def attention_pair(t):
            oT = [
                psum.tile([DA, L], fp32, tag="oT", bufs=2, name=f"oT_{t}_{hh}")
                for hh in range(2)
            ]
            for jt in range(8):
                # overlap the next pair's qT/kT projection, one 4-matmul
                # chunk at a time (1 of 4 work slots, never starves scores)
                if t < 3 and 2 <= jt <= 5:
                    proj_qk_chunk(t + 1, (jt - 2) // 2, jt % 2)
                j0 = jt * 128
                for hh in range(2):
                    h = 2 * t + hh
                    pb = 64 * hh  # partition base of this head inside tile t
                    # one [128, <=512] score chunk per oT PSUM bank (ih)
                    for ih in range(2):
                        lo, hi = max(j0, ih * 512), (ih + 1) * 512
                        if lo >= hi:
                            continue
                        cw = hi - lo
                        ps = psum.tile(
                            [128, 512], fp32, tag="work", bufs=4,
                            name=f"ps_{t}_{jt}_{hh}_{ih}",
                        )
                        # S^T chunk: [key j0..j0+128) x query lo..hi)
                        nc.tensor.matmul(
                            ps[:, :cw],
                            kT_sb[pb:pb + 64, t, j0:j0 + 128],
                            qT_sb[pb:pb + 64, t, lo:hi],
                            start=True,
                            stop=True,
                        )
                        pexp = ppool.tile(
                            [128, 512], bf16, tag="P", name=f"P_{t}_{jt}_{hh}_{ih}"
                        )
                        nc.scalar.activation(pexp[:, :cw], ps[:, :cw], EXP)
                        if lo == j0:
                            # causal mask inside the diagonal 128x128 block
                            nc.vector.tensor_mul(
                                pexp[:, 0:128], pexp[:, 0:128], mask_sb[:]
                            )
                        # O^T[:, lo:hi] += [v_h | 1].T @ P
                        nc.tensor.matmul(
                            oT[hh][:, lo:hi],
                            v_sb[:, jt, h, :],
                            pexp[:, :cw],
                            start=(jt == 0),
                            stop=(jt == (3 if ih == 0 else 7)),
                            skip_group_check=True,
                        )
            for hh in range(2):
                o_st = stage.tile([DA, L], fp32, tag="ost", name=f"ost_{t}_{hh}")
                nc.vector.tensor_copy(out=o_st[:], in_=oT[hh][:])
                nc.sync.dma_start(out[2 * t + hh], o_st[:])

        # emit: pair-0 dependencies first so the ScalarE exp stream (the
        # critical resource) starts as early as possible
        for it in range(8):
            proj_v(it)
        proj_qk(0)
        if variant == "proj":
            for t in range(1, 4):
                proj_qk(t)
            for h in range(8):
                o_st = stage.tile([DA, L], fp32, tag="ost", name=f"ostp_{h}")
                nc.vector.tensor_copy(out=o_st[:], in_=qT_sb[0:DA, h % 4, :])
                nc.sync.dma_start(out[h], o_st[:])
            return
        for t in range(4):
            attention_pair(t)


def _build_graph():
    import concourse.mybir as mybir
    import concourse.tile as tile
    from concourse import bacc

    nc = bacc.Bacc("TRN2", target_bir_lowering=False)
    bf16 = mybir.dt.bfloat16
    fp32 = mybir.dt.float32
    qsT = nc.dram_tensor("QsT", (D_IN, L), bf16, kind="ExternalInput")
    ksT = nc.dram_tensor("KsT", (D_IN, L), bf16, kind="ExternalInput")
    vsT = nc.dram_tensor("VsT", (D_IN, L), bf16, kind="ExternalInput")
    wq = nc.dram_tensor("WQ", (D_IN, H * D), bf16, kind="ExternalInput")
    wk = nc.dram_tensor("WK", (D_IN, H * D), bf16, kind="ExternalInput")
    wv = nc.dram_tensor("WV", (D_IN, H * D), bf16, kind="ExternalInput")
    mask = nc.dram_tensor("MASK", (128, 128), bf16, kind="ExternalInput")
    out = nc.dram_tensor("OUT", (H, DA, L), fp32, kind="ExternalOutput")

    with tile.TileContext(nc) as tc:
        build_attention_body(
            tc, qsT[:], ksT[:], vsT[:], wq[:], wk[:], wv[:], mask[:], out[:]
        )
    nc.compile()
    return nc


def get_graph():
    if "nc" not in _GRAPH_CACHE:
        _GRAPH_CACHE["nc"] = _build_graph()
    return _GRAPH_CACHE["nc"]


def make_in_maps(Q_seq, K_seq, V_seq, WQ, WK, WV):
    bf = ml_dtypes.bfloat16
    # fold the softmax 1/sqrt(D) into WQ so no scale is needed on-device
    wq = (np.asarray(WQ, dtype=np.float32) * SCALE).astype(bf)
    wk = np.asarray(WK, dtype=np.float32).astype(bf)
    wv = np.asarray(WV, dtype=np.float32).astype(bf)
    # keep-mask in S^T block coords: row r = key offset, col c = query offset;
    # keep key <= query  <=>  r <= c  (upper triangular incl. diagonal)
    mask = np.triu(np.ones((128, 128), dtype=np.float32)).astype(bf)
    in_maps = []
    for b in range(N_CORES):
        in_maps.append({
            "QsT": np.ascontiguousarray(np.asarray(Q_seq[b], np.float32).T).astype(bf),
            "KsT": np.ascontiguousarray(np.asarray(K_seq[b], np.float32).T).astype(bf),
            "VsT": np.ascontiguousarray(np.asarray(V_seq[b], np.float32).T).astype(bf),
            "WQ": wq,
            "WK": wk,
            "WV": wv,
            "MASK": mask,
        })
    return in_maps


def unshard(results):
    """results: list of per-core {"OUT": [H, DA, L] f32} -> [B, L, H*D] f32."""
    outs = np.stack([r["OUT"] for r in results])        # [B, H, DA, L]
    o = outs[:, :, :D, :] / outs[:, :, D:D + 1, :]       # [B, H, D, L]
    return np.ascontiguousarray(
        o.transpose(0, 3, 1, 2).reshape(B, L, H * D)
    ).astype(np.float32)


def run(inputs, **run_kwargs):
    """Compile + run on the 8 cores; returns (output, BassKernelResults)."""
    from concourse.bass_utils import run_bass_kernel_spmd

    nc = get_graph()
    in_maps = make_in_maps(
        inputs["Q_seq"], inputs["K_seq"], inputs["V_seq"],
        inputs["WQ"], inputs["WK"], inputs["WV"],
    )
    res = run_bass_kernel_spmd(
        nc, in_maps, core_ids=list(range(N_CORES)), **run_kwargs
    )
    return unshard(res.results), res


def kernel(Q_seq, K_seq, V_seq, WQ, WK, WV):
    out, _ = run({
        "Q_seq": Q_seq, "K_seq": K_seq, "V_seq": V_seq,
        "WQ": WQ, "WK": WK, "WV": WV,
    })
    return out
